# revision 34
# baseline (speedup 1.0000x reference)
"""BoundaryMaxPooling Trainium2 kernel.

Reference computation (B=16, C2=512, T=Tf=126):
  - segment windows [s0,s1) / [e0,e1) derived from segments[0] only (batch-0 row)
  - out[b, c, t]      = max_{j in [s0(t), s1(t))} feature[b, c, j]       (c < 256)
  - out[b, 256+c, t]  = max_{j in [e0(t), e1(t))} feature[b, 256+c, j]

Device algorithm (per core, 2 batches, data-parallel over batch):
  Sparse-table (log-level) range max with time j on SBUF partitions and
  c' = half*512 + local_batch*256 + channel on the free dim (1024 cols, bf16):
    L_0 = feature^T;  L_{k+1}[j] = max(L_k[j], L_k[j + 2^k])
  Shifts for 2^k in {1,2,4,8,16,32} are PE matmuls against an exact one-hot
  band matrix (fp8 - exact 0/1); the result lands in PSUM and the DVE maxes
  it with L_k into L_{k+1} (bf16, SBUF).  (Compute engines cannot read SBUF
  at a partition offset, so every shift must go through the PE.)
  Window max for window length L, k = floor(log2 L):
    out[t] = max(L_k[a(t)], L_k[b(t)]),  a = lo, b = hi - 2^k
  Both lookups are exact one-hot gather matmuls (fp8 weights x bf16 moving)
  accumulated over levels in PSUM; a zero one-hot column contributes exact 0.
  Host precomputes all index matrices from segments[0] (replicated across
  cores), pre-transposes features per core (bf16), and reassembles the bf16
  output; empty end-windows are set to float32 min on the host, matching
  the reference.
"""

import os
import sys

import numpy as np
import ml_dtypes

if os.path.isdir("/opt/trn_rl_repo") and "/opt/trn_rl_repo" not in sys.path:
    sys.path.insert(0, "/opt/trn_rl_repo")

import concourse.bass as bass  # noqa: E402
from concourse import bacc, mybir, tile  # noqa: E402
from concourse.bass_utils import run_bass_kernel_spmd  # noqa: E402

B, C2, T = 16, 512, 126
C = C2 // 2  # 256
NCORES = 8
BPC = B // NCORES  # batches per core = 2
CPRIME = BPC * C2  # 1024 columns per core
NLEV = 7
KS = [127 - (1 << k) for k in range(NLEV)]  # valid rows of level k
PE_LEVELS = 6  # shifts 1..32 all on the PE (DVE cannot offset partitions)

F32 = mybir.dt.float32
BF16 = mybir.dt.bfloat16
FP8 = mybir.dt.float8e4
MAX = mybir.AluOpType.max

NP_BF16 = ml_dtypes.bfloat16
NP_FP8 = ml_dtypes.float8_e4m3

_CACHE = {}

# test.py hooks: set TRACE=True before calling kernel() to capture a profile.
TRACE = False
LAST_RESULTS = None


def _oh_layout():
    """fp8 one-hot weights, three chunks split by need-time.

    chunk0: level 0 (gates the first shift; small + fast scalar queue)
    chunk1: levels 1-2 (second DMA on the sync queue, behind the feature)
    chunk2: levels 3-6 (slow gpsimd SWDGE path; needed ~3 levels in)
    Returns ({key: (chunk, off, n)}, [chunk_total, ...]).
    """
    offs = {}
    totals = []
    chunk_of_level = [0, 1, 1, 2, 2, 2, 2]
    off = 0
    cur = 0
    for k in range(NLEV):
        ch = chunk_of_level[k]
        if ch != cur:
            totals.append(off)
            cur = ch
            off = 0
        if k < PE_LEVELS:
            offs[("sh", k)] = (ch, off, KS[k + 1])
            off += KS[k + 1]
        for gi in range(2):
            for h in range(2):
                offs[("g", gi, h, k)] = (ch, off, T)
                off += T
    totals.append(off)
    return offs, totals


def _build_module():
    nc = bacc.Bacc(None, target_bir_lowering=False, debug=False)

    offs, totals = _oh_layout()
    inp_f = nc.dram_tensor("inpf", [T, CPRIME], BF16, kind="ExternalInput")
    inp_oh = [
        nc.dram_tensor(f"inpoh{c}", [T, totals[c]], FP8, kind="ExternalInput")
        for c in range(3)
    ]
    out = nc.dram_tensor("out", [T, CPRIME], BF16, kind="ExternalOutput")

    with tile.TileContext(nc) as tc:
        with (
            tc.tile_pool(name="lv", bufs=1) as lvp,
            tc.tile_pool(name="gw", bufs=1) as gwp,
            tc.tile_pool(name="acc", bufs=1, space=bass.MemorySpace.PSUM) as accp,
            tc.tile_pool(name="shp", bufs=3, space=bass.MemorySpace.PSUM) as shpp,
        ):
            # wjunk feeds the PE warmup; memset it first on the gpsimd queue
            # (free earliest) so the warmup starts right after the PE
            # preamble.
            wjunk = gwp.tile([128, 512], BF16, name="wjunk")
            nc.gpsimd.memset(wjunk[:, :], 0.0)

            # HWDGE queues (sync/scalar) have ~2us less latency than the
            # gpsimd SWDGE path.  The gate for the first shift is
            # max(feature, level-0 weights): feature rides sync, the small
            # level-0 chunk rides scalar, level-1 follows the feature on
            # sync, and levels 2-6 take the slow gpsimd path (needed ~2
            # levels in).
            ft = gwp.tile([T, CPRIME], BF16, name="ft")
            oh = [gwp.tile([T, totals[c]], FP8, name=f"oh{c}") for c in range(3)]
            nc.sync.dma_start(out=ft[:, :], in_=inp_f[:, :])
            nc.scalar.dma_start(out=oh[0][:, :], in_=inp_oh[0][:, :])
            nc.sync.dma_start(out=oh[1][:, :], in_=inp_oh[1][:, :])
            nc.gpsimd.dma_start(out=oh[2][:, :], in_=inp_oh[2][:, :])

            L = [ft[:, 0:CPRIME]] + [
                lvp.tile([KS[k], CPRIME], BF16, name=f"L{k}")[:, :]
                for k in range(1, NLEV)
            ]

            def sh_ap(k):
                ch, o, n = offs[("sh", k)]
                return oh[ch][0 : KS[k], o : o + n]

            def g_ap(gi, h, k):
                ch, o, n = offs[("g", gi, h, k)]
                return oh[ch][0 : KS[k], o : o + n]

            # Per-(gi, half) accumulators: separate tiles keep the h0 stop
            # events independent of h1's, so the h0 final combine can start a
            # full macro-step before h1's last gathers retire.
            p_acc = [
                [accp.tile([T, 512], F32, name=f"pacc{gi}h{h}") for h in range(2)]
                for gi in range(2)
            ]

            # PE warmup: HAM throttles the PE to half clock until it has been
            # continuously busy ~3.4us. Burn dummy matmuls on a never-written
            # tile (garbage values, results discarded by the later start=True
            # accumulation reset) while the input DMAs land, so the real
            # matmuls run at full clock with no dependency on any engine.
            shp_warm = shpp.tile([KS[1], 512], F32, name="shwarm", tag="shp")
            for w in range(6):
                nc.tensor.matmul(
                    shp_warm[0:125, 0:512],
                    wjunk[0:128, 0:125],
                    wjunk[0:128, 0:512],
                    start=True,
                    stop=True,
                )

            # Skewed per-half pipeline: h0 runs level k while h1 runs level
            # k-1.  By the time the DVE retires one half's max, the other
            # half's shift (issued a full level earlier) has long landed, so
            # the DVE runs back-to-back instead of waiting on the PE->PSUM
            # drain each level.
            def half_level(h, k):
                sl = slice(h * 512, (h + 1) * 512)
                if k < PE_LEVELS:
                    shp = shpp.tile(
                        [KS[k + 1], 512], F32, name=f"shp{k}h{h}", tag="shp"
                    )
                    nc.tensor.matmul(
                        shp[:, :],
                        sh_ap(k),
                        L[k][:, sl],
                        start=True,
                        stop=True,
                    )
                    nc.vector.tensor_max(
                        L[k + 1][0 : KS[k + 1], sl],
                        L[k][0 : KS[k + 1], sl],
                        shp[:, :],
                    )
                for gi in range(2):
                    nc.tensor.matmul(
                        p_acc[gi][h][:, :],
                        g_ap(gi, h, k),
                        L[k][:, sl],
                        start=(k == 0),
                        stop=(k == NLEV - 1),
                    )

            for k in range(NLEV + 1):
                if k < NLEV:
                    half_level(0, k)
                if k >= 1:
                    half_level(1, k - 1)


            s1t = gwp.tile([T, CPRIME], BF16, name="s1t")
            ot = gwp.tile([T, CPRIME], BF16, name="ot")
            for h in range(2):
                sl = slice(h * 512, (h + 1) * 512)
                nc.scalar.copy(out=s1t[:, sl], in_=p_acc[0][h][:, :])
                nc.vector.tensor_max(ot[:, sl], s1t[:, sl], p_acc[1][h][:, :])
                eng = nc.sync if h == 0 else nc.scalar
                eng.dma_start(out=out[:, sl], in_=ot[:, sl])

    nc.compile()
    return nc


def _host_windows(segments):
    """Replicates the reference's index math on segments[0]. Returns per half
    (lo, hi) clamped windows plus the empty mask."""
    seg = np.clip(segments.astype(np.float32), 0.0, 125.0)
    row = seg[0]  # [T, 4]
    s0 = np.floor(row[:, 0]).astype(np.int32)
    s1 = np.ceil(row[:, 1]).astype(np.int32)
    s1 = np.where(s0 == s1, s1 + 1, s1)
    e0 = np.floor(row[:, 2]).astype(np.int32)
    e1 = np.ceil(row[:, 3]).astype(np.int32)
    e0 = np.where(e0 == e1, e0 - 1, e0)

    halves = []
    for lo, hi in ((s0, s1), (e0, e1)):
        lo_c = np.maximum(lo, 0)
        hi_c = np.minimum(hi, T)
        empty = lo_c >= hi_c
        halves.append((lo_c, hi_c, empty))
    return halves


def _host_matrices(segments):
    halves = _host_windows(segments)
    g = {
        (gi, h, k): np.zeros((KS[k], T), np.float32)
        for gi in range(2)
        for h in range(2)
        for k in range(NLEV)
    }
    for h, (lo, hi, empty) in enumerate(halves):
        for t in range(T):
            if empty[t]:
                continue
            ln = int(hi[t] - lo[t])
            k = ln.bit_length() - 1
            a = int(lo[t])
            b = int(hi[t]) - (1 << k)
            g[(0, h, k)][a, t] = 1.0
            g[(1, h, k)][b, t] = 1.0
    sh = {}
    for k in range(PE_LEVELS):
        m = np.zeros((KS[k], KS[k + 1]), np.float32)
        s = 1 << k
        for j in range(KS[k + 1]):
            m[j + s, j] = 1.0
        sh[k] = m
    return g, sh, halves


def _shard_feature(feature):
    """Core i gets batches [2i, 2i+2) as bf16 [T, CPRIME] with
    c' = half*512 + local_batch*256 + channel_within_half."""
    fts = []
    for i in range(NCORES):
        pair = np.ascontiguousarray(feature[BPC * i : BPC * (i + 1)])
        arr = pair.reshape(BPC, 2, C, T)  # [b, h, c, j]
        arr = np.ascontiguousarray(arr.transpose(3, 1, 0, 2).reshape(T, CPRIME))
        fts.append(arr.astype(NP_BF16))
    return fts


def _unshard(results, halves):
    out = np.empty((B, C2, T), np.float32)
    for i in range(NCORES):
        r = np.asarray(results[i]["out"]).astype(np.float32)  # [T, CPRIME]
        arr = r.reshape(T, 2, BPC, C).transpose(2, 1, 3, 0)  # [b, h, c, t]
        out[BPC * i : BPC * (i + 1)] = arr.reshape(BPC, C2, T)
    neg = np.finfo(np.float32).min
    for h, (_, _, empty) in enumerate(halves):
        if empty.any():
            out[:, h * C : (h + 1) * C, empty] = neg
    return out


def kernel(feature, segments):
    global LAST_RESULTS
    feature = np.ascontiguousarray(feature, dtype=np.float32)
    segments = np.ascontiguousarray(segments, dtype=np.float32)

    if "nc" not in _CACHE:
        _CACHE["nc"] = _build_module()
    nc = _CACHE["nc"]

    g, sh, halves = _host_matrices(segments)
    fts = _shard_feature(feature)

    offs, totals = _oh_layout()
    chunks = [np.zeros((T, totals[c]), np.float32) for c in range(2)]
    for k in range(NLEV):
        if k < PE_LEVELS:
            ch, o, n = offs[("sh", k)]
            chunks[ch][: KS[k], o : o + n] = sh[k]
        for gi in range(2):
            for h in range(2):
                ch, o, n = offs[("g", gi, h, k)]
                chunks[ch][: KS[k], o : o + n] = g[(gi, h, k)]
    chunks = [c.astype(NP_FP8) for c in chunks]
    in_maps = []
    for i in range(NCORES):
        in_maps.append(
            {
                "inpf": fts[i],
                "inpoh0": chunks[0],
                "inpoh1": chunks[1],
                "inpoh2": chunks[2],
            }
        )

    res = run_bass_kernel_spmd(nc, in_maps, list(range(NCORES)), trace=TRACE)
    LAST_RESULTS = res
    return _unshard(res.results, halves)
